# revision 40
# baseline (speedup 1.0000x reference)
"""AreaAttention Trainium2 kernel: B=8 data-parallel over 8 NeuronCores.

Reference computation (per sample, C=128 channels, N=H*W=4096 pixels):
    q = Wq@x + bq                    ('oc,bcn->bno' proper matmul)
    k = x * colsum(Wk) + bk          ('oc,bcn->bcn' keeps c: per-channel scale!)
    v = Wv@x + bv                    ('oc,bcn->bno')
    out = x + softmax(q^T k / sqrt(C)) @ v^T

Per-core design (one sample per core, no collectives):
  - q/k/v all fp8e4m3. q8z [c, 3, n]: plane1 = q8, planes 0/2 = zeros.
  - QK via zero-padded fp8 DoubleRow (0.5 cyc/out-col): stationary
    k8[:, 2jp:2jp+2, :], moving [q8, 0] (u=0) or [0, q8] (u=1).
  - exp(s/sqrt(C) - 4) -> fp8 es, split ACT (exact table exp) / DVE
    (bit-hack: u8 = sat(s*a + b) IS the e4m3 bit pattern of 2^affine(s)
    -- a single tensor_scalar). GPSIMD cannot read PSUM, so it takes the
    SBUF-only work instead (memsets, mid-block residual adds).
  - PV: one DoubleRow fp8 matmul per pair (v8 [m,2,c] stationary).
  - Denominator: DoubleRow matmul vs all-ones stationary -> psum [128,512]
    broadcast, accumulated over pairs. No DVE chain adds.
  - PV/dn are emitted LAG pairs behind QK/exp so the in-order PE never
    stalls on a DVE-queued exp.
  - PSUM: sc 3x[128,2,512] (6 banks) + pv [128,512] + dn [128,512] = 8.
"""
import numpy as np

C = 128
N = 4096          # 64*64
NB = 512          # n-block span
NBLK = N // NB    # 8
MCH = N // C      # 32 m-chunks
NPAIR = MCH // 2  # 16 chunk-pairs per block
SCALE = 1.0 / np.sqrt(np.float32(C))
ESHIFT = -4.0
LOG2E = 1.4426950408889634
# bit-hack exp: u8 bits of e4m3(2^((s*SCALE+ESHIFT)*log2e)) = s*BH_A + BH_B
BH_A = 8.0 * SCALE * LOG2E
BH_B = 8.0 * (7.0 + ESHIFT * LOG2E)
NHOIST = 4
# PV/dn trail QK/exp by LAG pairs, so the in-order PE never stalls on a
# fresh (possibly DVE-queued) exp, and the previous block's tail (recip+
# mul reading the single-buffered pv/dn psum) drains before the next
# block's first PV hits the WAR on those banks.
LAG = 8

# per-block DVE exp pairs (rest on ACT). pairs 0-3 stay on ACT (hoisted
# across block boundaries; DVE there would delay the prior block's tail).
# pairs 13-15 also stay on ACT (their sc-psum bufs feed the hoisted QKs).
# NEVER cluster DVE pairs: QK(jp) waits exp(jp-3) via the sc rotation, so
# back-to-back DVE exps stall the QK stream and starve ACT.
DV0 = {5, 9}                          # block 0 (setup-heavy on DVE)
DV = {4, 7, 10, 12}                   # blocks 1..6
DV7 = {4, 7, 10}                      # last block: DVE free early

_cache = {}


def _build_nc():
    import concourse.tile as tile
    from concourse import bacc, mybir

    f32 = mybir.dt.float32
    f16 = mybir.dt.float16
    f8 = mybir.dt.float8e4
    u8 = mybir.dt.uint8
    ADD = mybir.AluOpType.add
    MUL = mybir.AluOpType.mult
    EXP = mybir.ActivationFunctionType.Exp
    DR = mybir.MatmulPerfMode.DoubleRow

    nc = bacc.Bacc("TRN2", target_bir_lowering=False)

    x_d = nc.dram_tensor("x", [C, N], f32, kind="ExternalInput")
    wqt16_d = nc.dram_tensor("wqt16", [C, C], f16, kind="ExternalInput")
    wvt16_d = nc.dram_tensor("wvt16", [C, C], f16, kind="ExternalInput")
    w4_d = nc.dram_tensor("w4", [C, 4], f32, kind="ExternalInput")  # wks|bk|bq|bv
    out_d = nc.dram_tensor("out", [C, N], f32, kind="ExternalOutput")

    SLICE = 1024  # setup granularity (4 slices)

    with tile.TileContext(nc) as tc:
        with tc.tile_pool(name="big", bufs=1) as big, \
             tc.tile_pool(name="small", bufs=1) as small, \
             tc.tile_pool(name="es_pool", bufs=16) as es_pool, \
             tc.tile_pool(name="work", bufs=2) as work, \
             tc.tile_pool(name="ps_sc", bufs=3, space="PSUM") as ps_sc, \
             tc.tile_pool(name="ps_pv", bufs=1, space="PSUM") as ps_pv, \
             tc.tile_pool(name="ps_dn", bufs=1, space="PSUM") as ps_dn:

            xfb = big.tile([C, N], f32, tag="xfb")      # x, then x + bv (residual)
            xf16 = big.tile([C, N], f16, tag="xf16")    # x fp16 (q/v proj, k build)
            q8z = big.tile([C, 3, N], f8, tag="q8z")    # plane1=q8, planes 0/2=0
            k8 = big.tile([C, MCH, C], f8, tag="k8")    # [c, chunk, m]
            v8 = big.tile([C, MCH, C], f8, tag="v8")    # [m-within, chunk, c]

            wqt16 = small.tile([C, C], f16, tag="wqt16")
            wvt16 = small.tile([C, C], f16, tag="wvt16")
            w4 = small.tile([C, 4], f32, tag="w4")
            ebias = small.tile([C, 1], f32, tag="ebias")
            ones8 = small.tile([C, 2, C], f8, tag="ones8")
            wks, bk, bq, bv = (w4[:, i:i + 1] for i in range(4))

            # first 512 x-cols split in quarters across both DMA queues so
            # the QK-pair-0 path unblocks as early as possible; the single
            # w4 DMA carries every bias/scale vector and lands first.
            nc.sync.dma_start(xfb[:, 0:256], x_d[:, 0:256])
            nc.scalar.dma_start(w4[:], w4_d[:])
            nc.scalar.dma_start(xfb[:, 256:512], x_d[:, 256:512])
            nc.sync.dma_start(xfb[:, 512:SLICE], x_d[:, 512:SLICE])
            nc.scalar.dma_start(wqt16[:], wqt16_d[:])
            for s in range(1, 4):
                sl = slice(s * SLICE, (s + 1) * SLICE)
                nc.sync.dma_start(xfb[:, sl], x_d[:, sl])
            nc.scalar.dma_start(wvt16[:], wvt16_d[:])
            nc.gpsimd.memset(ebias[:], ESHIFT)
            nc.gpsimd.memset(ones8[:], 1.0)
            # DVE warm-up: the first few DVE ops otherwise run 3-4x slow
            # (cold clock); burn ~3us on a scratch region that setup_qk
            # slices 1-3 will overwrite much later
            nc.vector.memset(xf16[:, 1024:N], 0.0)
            # zero planes for blocks 0-3 now; blocks 4-7 later (gpsimd slack)
            nc.gpsimd.memset(q8z[:, 2, 0:2048], 0.0)
            nc.gpsimd.memset(q8z[:, 0, 0:2048], 0.0)

            def setup_qk(s, half=None):
                """xf16 cast, k8, q8 for one 1024-col slice (or a 512 half).
                cast/k8/resid on GPSIMD (SBUF-only); q8 bias-add on DVE (PSUM)."""
                lo = s * SLICE + (0 if half in (None, 0) else 512)
                w = SLICE if half is None else 512
                sl = slice(lo, lo + w)
                nch = w // C
                j0 = lo // C
                nc.vector.tensor_copy(xf16[:, sl], xfb[:, sl])
                with nc.allow_low_precision(reason="fp8 attention, tol 2e-2"):
                    nc.vector.tensor_scalar(k8[:, j0:j0 + nch, :], xf16[:, sl],
                                            wks, bk, op0=MUL, op1=ADD)
                    ps = ps_sc.tile([C, 2, 512], f32, tag="sc", name=f"qps{s}_{half}")
                    for h in range(w // 512):
                        hsl = slice(lo + h * 512, lo + (h + 1) * 512)
                        nc.tensor.matmul(ps[:, h, :], wqt16[:], xf16[:, hsl],
                                         start=True, stop=True)
                    nc.vector.tensor_scalar(q8z[:, 1, sl], ps[:, 0:w // 512, :],
                                            bq, None, op0=ADD)

            def resid(s):
                """xfb += bv for one slice (after the slice's cast read xfb)."""
                sl = slice(s * SLICE, (s + 1) * SLICE)
                nc.vector.tensor_scalar(xfb[:, sl], xfb[:, sl], bv, None,
                                        op0=ADD)

            def setup_v(s):
                """v8 chunks for one 1024-col slice (8 chunks, one psum tile)."""
                psv = ps_sc.tile([C, 2, 512], f32, tag="sc", name=f"vps{s}")
                for t in range(8):
                    j = s * 8 + t
                    nc.tensor.matmul(psv[:, t // 4, (t % 4) * C:(t % 4 + 1) * C],
                                     xf16[:, j * C:(j + 1) * C], wvt16[:],
                                     start=True, stop=True)
                with nc.allow_low_precision(reason="fp8 attention, tol 2e-2"):
                    nc.vector.tensor_copy(v8[:, s * 8:(s + 1) * 8, :], psv[:, :, :])

            def emit_qk_exp(nb, jp, on_dve):
                """QK DoubleRow matmuls + exp for one chunk-pair."""
                n0 = nb * NB
                sc = ps_sc.tile([C, 2, 512], f32, tag="sc", name=f"sc{nb}_{jp}")
                for u in range(2):
                    # moving planes: u=0 -> [q8, 0], u=1 -> [0, q8]
                    nc.tensor.matmul(sc[:, u, :], k8[:, 2 * jp:2 * jp + 2, :],
                                     q8z[:, 1 - u:3 - u, n0:n0 + NB],
                                     start=True, stop=True, perf_mode=DR)
                es = es_pool.tile([C, 2, 512], f8, tag="es", name=f"es{nb}_{jp}")
                with nc.allow_low_precision(reason="fp8 attention, tol 2e-2"):
                    if on_dve:
                        nc.vector.tensor_scalar(es[:, :, :].bitcast(u8),
                                                sc[:, :, :], float(BH_A),
                                                float(BH_B), op0=MUL, op1=ADD)
                    else:
                        nc.scalar.activation(es[:, :, :], sc[:, :, :], EXP,
                                             bias=ebias[:], scale=float(SCALE))
                return es

            def emit_pv_dn(nb, jp, pv, dn, es):
                nc.tensor.matmul(pv[:], v8[:, 2 * jp:2 * jp + 2, :], es[:, :, :],
                                 start=(jp == 0), stop=(jp == NPAIR - 1),
                                 perf_mode=DR)
                nc.tensor.matmul(dn[:], ones8[:, :, :], es[:, :, :],
                                 start=(jp == 0), stop=(jp == NPAIR - 1),
                                 perf_mode=DR)

            def tail(nb, pv, dn):
                osl = slice(nb * NB, (nb + 1) * NB)
                rb = work.tile([C, NB], f32, tag="rb", name=f"rb{nb}")
                ep1 = work.tile([C, NB], f32, tag="ep1", name=f"ep{nb}")
                ost = work.tile([C, NB], f32, tag="ost", name=f"ost{nb}")
                if nb == NBLK - 1:
                    # final tail: no successor needs the psum banks, so skip
                    # the pv drain and pipeline the whole normalize + DMA in
                    # 256-col halves (first half's DMA overlaps second half)
                    for h in (slice(0, 256), slice(256, NB)):
                        o2 = slice(nb * NB + h.start, nb * NB + h.stop)
                        nc.vector.reciprocal_approx_fast(rb[:, h], dn[:, h])
                        nc.vector.tensor_tensor(ep1[:, h], pv[:, h], rb[:, h],
                                                op=MUL)
                        nc.vector.tensor_tensor(ost[:, h], ep1[:, h],
                                                xfb[:, o2], op=ADD)
                        nc.sync.dma_start(out_d[:, o2], ost[:, h])
                    return
                pvs = work.tile([C, NB], f32, tag="pvs", name=f"pvs{nb}")
                # drain pv to SBUF FIRST: unlike the recip->mul chain it has
                # no dependencies, so the next block's first PV (WAR on the
                # single pv psum bank) unblocks ~1us sooner
                nc.vector.tensor_copy(pvs[:], pv[:])
                nc.vector.reciprocal_approx_fast(rb[:], dn[:])
                nc.vector.tensor_tensor(ep1[:], pvs[:], rb[:], op=MUL)
                nc.gpsimd.tensor_tensor(ost[:], ep1[:], xfb[:, osl], op=ADD)
                nc.sync.dma_start(out_d[:, osl], ost[:])

            # quarter-granular first setup: k8 chunks 0-3 land pair-0/1
            # stationaries the moment each 256-col x quarter arrives
            with nc.allow_low_precision(reason="fp8 attention, tol 2e-2"):
                nc.vector.tensor_copy(xf16[:, 0:256], xfb[:, 0:256])
                nc.vector.tensor_scalar(k8[:, 0:2, :], xf16[:, 0:256],
                                        wks, bk, op0=MUL, op1=ADD)
                nc.vector.tensor_copy(xf16[:, 256:512], xfb[:, 256:512])
                nc.vector.tensor_scalar(k8[:, 2:4, :], xf16[:, 256:512],
                                        wks, bk, op0=MUL, op1=ADD)
                ps0 = ps_sc.tile([C, 2, 512], f32, tag="sc", name="qps0_0")
                nc.tensor.matmul(ps0[:, 0, :], wqt16[:], xf16[:, 0:512],
                                 start=True, stop=True)
                nc.vector.tensor_scalar(q8z[:, 1, 0:512], ps0[:, 0:1, :],
                                        bq, None, op0=ADD)
            setup_qk(0, half=1)

            hoisted = {}
            for nb in range(NBLK):
                dv = DV0 if nb == 0 else (DV7 if nb == NBLK - 1 else DV)
                pv = ps_pv.tile([C, NB], f32, tag="pv", name=f"pv{nb}")
                dn = ps_dn.tile([C, NB], f32, tag="dn", name=f"dn{nb}")
                pending = []

                for jp in range(NPAIR):
                    # spread the projection setups so no single burst starves
                    # the QK->exp stream (qk at 2/6/10, v at 4/8/12)
                    if nb == 0 and jp in (2, 6, 10):
                        setup_qk((jp + 2) // 4)  # slices 1, 2, 3
                        if jp == 2:  # zero planes for blocks 4-7
                            nc.gpsimd.memset(q8z[:, 2, 2048:N], 0.0)
                            nc.gpsimd.memset(q8z[:, 0, 2048:N], 0.0)
                    if nb == 0 and jp in (4, 8, 12):
                        setup_v(jp // 4)
                        resid(jp // 4)
                    es = hoisted.pop((nb, jp), None)
                    if es is None:
                        es = emit_qk_exp(nb, jp, jp in dv)
                    if nb == 0 and jp == 0:
                        setup_v(0)
                        resid(0)
                    pending.append((jp, es))
                    if jp == NPAIR - 1:
                        # boundary: flush all but the final pair (their exps
                        # are long done), hoist 2 QK/exps, flush the final
                        # pair (its exp has now had time to finish), then 2
                        # more hoists -- the PE never waits on a fresh exp
                        # and ACT stays fed through the tail
                        while len(pending) > 1:
                            pjp, pes = pending.pop(0)
                            emit_pv_dn(nb, pjp, pv, dn, pes)
                        if nb < NBLK - 1:
                            for hj in range(2):
                                hoisted[(nb + 1, hj)] = emit_qk_exp(nb + 1, hj, False)
                        pjp, pes = pending.pop(0)
                        emit_pv_dn(nb, pjp, pv, dn, pes)
                        if nb < NBLK - 1:
                            for hj in range(2, NHOIST):
                                hoisted[(nb + 1, hj)] = emit_qk_exp(nb + 1, hj, False)
                    elif len(pending) > LAG:
                        pjp, pes = pending.pop(0)
                        emit_pv_dn(nb, pjp, pv, dn, pes)

                tail(nb, pv, dn)

    nc.finalize()
    return nc


def _get_nc():
    if "nc" not in _cache:
        _cache["nc"] = _build_nc()
    return _cache["nc"]


def make_in_maps(x, Wq, bq, Wk, bk, Wv, bv):
    x = np.asarray(x, dtype=np.float32)
    B = x.shape[0]
    wqt16 = np.ascontiguousarray(np.asarray(Wq, np.float32).T).astype(np.float16)
    wvt16 = np.ascontiguousarray(np.asarray(Wv, np.float32).T).astype(np.float16)
    w4 = np.ascontiguousarray(np.stack([
        np.asarray(Wk, np.float32).sum(axis=0),
        np.asarray(bk, np.float32),
        np.asarray(bq, np.float32),
        np.asarray(bv, np.float32),
    ], axis=1))
    in_maps = []
    for i in range(B):
        in_maps.append({
            "x": np.ascontiguousarray(x[i].reshape(C, N)),
            "wqt16": wqt16, "wvt16": wvt16, "w4": w4,
        })
    return in_maps


def kernel(x, Wq, bq, Wk, bk, Wv, bv, _trace=False, _tmpdir=None):
    from concourse.bass_utils import run_bass_kernel_spmd

    x = np.asarray(x, dtype=np.float32)
    B, c, H, W = x.shape
    assert (c, H * W) == (C, N), (c, H, W)
    in_maps = make_in_maps(x, Wq, bq, Wk, bk, Wv, bv)
    nc = _get_nc()
    res = run_bass_kernel_spmd(nc, in_maps, core_ids=list(range(B)),
                               trace=_trace, tmpdir=_tmpdir)
    out = np.stack([res.results[i]["out"].reshape(C, H, W) for i in range(B)])
    if _trace:
        _cache["last_result"] = res
    return out.astype(np.float32)


# revision 41
# speedup vs baseline: 1.2001x; 1.2001x over previous
"""AreaAttention Trainium2 kernel: B=8 data-parallel over 8 NeuronCores.

Reference computation (per sample, C=128 channels, N=H*W=4096 pixels):
    q = Wq@x + bq                    ('oc,bcn->bno' proper matmul)
    k = x * colsum(Wk) + bk          ('oc,bcn->bcn' keeps c: per-channel scale!)
    v = Wv@x + bv                    ('oc,bcn->bno')
    out = x + softmax(q^T k / sqrt(C)) @ v^T

Per-core design (one sample per core, no collectives):
  - q/k/v all fp8e4m3. q8z [c, 3, n]: plane1 = q8, planes 0/2 = zeros.
  - QK via zero-padded fp8 DoubleRow (0.5 cyc/out-col): stationary
    k8[:, 2jp:2jp+2, :], moving [q8, 0] (u=0) or [0, q8] (u=1).
  - exp(s/sqrt(C) - 4) -> fp8 es, split ACT (exact table exp) / DVE
    (bit-hack: u8 = sat(s*a + b) IS the e4m3 bit pattern of 2^affine(s)
    -- a single tensor_scalar). GPSIMD cannot read PSUM, so it takes the
    SBUF-only work instead (memsets, mid-block residual adds).
  - PV: one DoubleRow fp8 matmul per pair (v8 [m,2,c] stationary).
  - Denominator: DoubleRow matmul vs all-ones stationary -> psum [128,512]
    broadcast, accumulated over pairs. No DVE chain adds.
  - PV/dn are emitted LAG pairs behind QK/exp so the in-order PE never
    stalls on a DVE-queued exp.
  - PSUM: sc 3x[128,2,512] (6 banks) + pv [128,512] + dn [128,512] = 8.
"""
import numpy as np

C = 128
N = 4096          # 64*64
NB = 512          # n-block span
NBLK = N // NB    # 8
MCH = N // C      # 32 m-chunks
NPAIR = MCH // 2  # 16 chunk-pairs per block
SCALE = 1.0 / np.sqrt(np.float32(C))
ESHIFT = -4.0
LOG2E = 1.4426950408889634
# bit-hack exp: u8 bits of e4m3(2^((s*SCALE+ESHIFT)*log2e)) = s*BH_A + BH_B
BH_A = 8.0 * SCALE * LOG2E
BH_B = 8.0 * (7.0 + ESHIFT * LOG2E)
NHOIST = 4
# PV/dn trail QK/exp by LAG pairs, so the in-order PE never stalls on a
# fresh (possibly DVE-queued) exp, and the previous block's tail (recip+
# mul reading the single-buffered pv/dn psum) drains before the next
# block's first PV hits the WAR on those banks.
LAG = 8

# per-block DVE exp pairs (rest on ACT). pairs 0-3 stay on ACT (hoisted
# across block boundaries; DVE there would delay the prior block's tail).
# pairs 13-15 also stay on ACT (their sc-psum bufs feed the hoisted QKs).
# NEVER cluster DVE pairs: QK(jp) waits exp(jp-3) via the sc rotation, so
# back-to-back DVE exps stall the QK stream and starve ACT.
DV0 = {5, 9, 12}                        # block 0 (setup-heavy on DVE)
DV = {4, 7, 10, 12}                   # blocks 1..6
DV7 = {4, 7, 10}                      # last block: DVE free early

_cache = {}


def _build_nc():
    import concourse.tile as tile
    from concourse import bacc, mybir

    f32 = mybir.dt.float32
    f16 = mybir.dt.float16
    f8 = mybir.dt.float8e4
    u8 = mybir.dt.uint8
    ADD = mybir.AluOpType.add
    MUL = mybir.AluOpType.mult
    EXP = mybir.ActivationFunctionType.Exp
    DR = mybir.MatmulPerfMode.DoubleRow

    nc = bacc.Bacc("TRN2", target_bir_lowering=False)

    x_d = nc.dram_tensor("x", [C, N], f32, kind="ExternalInput")
    wqt16_d = nc.dram_tensor("wqt16", [C, C], f16, kind="ExternalInput")
    wvt16_d = nc.dram_tensor("wvt16", [C, C], f16, kind="ExternalInput")
    w4_d = nc.dram_tensor("w4", [C, 4], f32, kind="ExternalInput")  # wks|bk|bq|bv
    out_d = nc.dram_tensor("out", [C, N], f32, kind="ExternalOutput")

    SLICE = 1024  # setup granularity (4 slices)

    with tile.TileContext(nc) as tc:
        with tc.tile_pool(name="big", bufs=1) as big, \
             tc.tile_pool(name="small", bufs=1) as small, \
             tc.tile_pool(name="es_pool", bufs=16) as es_pool, \
             tc.tile_pool(name="work", bufs=2) as work, \
             tc.tile_pool(name="ps_sc", bufs=3, space="PSUM") as ps_sc, \
             tc.tile_pool(name="ps_pv", bufs=1, space="PSUM") as ps_pv, \
             tc.tile_pool(name="ps_dn", bufs=1, space="PSUM") as ps_dn:

            xfb = big.tile([C, N], f32, tag="xfb")      # x, then x + bv (residual)
            xf16 = big.tile([C, N], f16, tag="xf16")    # x fp16 (q/v proj, k build)
            q8z = big.tile([C, 3, N], f8, tag="q8z")    # plane1=q8, planes 0/2=0
            k8 = big.tile([C, MCH, C], f8, tag="k8")    # [c, chunk, m]
            v8 = big.tile([C, MCH, C], f8, tag="v8")    # [m-within, chunk, c]

            wqt16 = small.tile([C, C], f16, tag="wqt16")
            wvt16 = small.tile([C, C], f16, tag="wvt16")
            w4 = small.tile([C, 4], f32, tag="w4")
            ebias = small.tile([C, 1], f32, tag="ebias")
            ones8 = small.tile([C, 2, C], f8, tag="ones8")
            wks, bk, bq, bv = (w4[:, i:i + 1] for i in range(4))

            # first 512 x-cols split in quarters across both DMA queues so
            # the QK-pair-0 path unblocks as early as possible; the single
            # w4 DMA carries every bias/scale vector and lands first.
            nc.sync.dma_start(xfb[:, 0:256], x_d[:, 0:256])
            nc.scalar.dma_start(w4[:], w4_d[:])
            nc.scalar.dma_start(xfb[:, 256:512], x_d[:, 256:512])
            nc.sync.dma_start(xfb[:, 512:SLICE], x_d[:, 512:SLICE])
            nc.scalar.dma_start(wqt16[:], wqt16_d[:])
            for s in range(1, 4):
                sl = slice(s * SLICE, (s + 1) * SLICE)
                nc.sync.dma_start(xfb[:, sl], x_d[:, sl])
            nc.scalar.dma_start(wvt16[:], wvt16_d[:])
            nc.gpsimd.memset(ebias[:], ESHIFT)
            nc.gpsimd.memset(ones8[:], 1.0)
            # DVE warm-up: the first few DVE ops otherwise run 3-4x slow
            # (cold clock); burn ~3us on a scratch region that setup_qk
            # slices 1-3 will overwrite much later
            nc.vector.memset(xf16[:, 1024:N], 0.0)
            # zero planes for blocks 0-3 now; blocks 4-7 later (gpsimd slack)
            nc.gpsimd.memset(q8z[:, 2, 0:2048], 0.0)
            nc.gpsimd.memset(q8z[:, 0, 0:2048], 0.0)

            def setup_k(s):
                """xf16 cast + k8 for one 1024-col slice (every block needs
                all key chunks, so these must complete within block 0)."""
                sl = slice(s * SLICE, (s + 1) * SLICE)
                nc.vector.tensor_copy(xf16[:, sl], xfb[:, sl])
                with nc.allow_low_precision(reason="fp8 attention, tol 2e-2"):
                    nc.vector.tensor_scalar(k8[:, s * 8:(s + 1) * 8, :],
                                            xf16[:, sl], wks, bk,
                                            op0=MUL, op1=ADD)

            def setup_q(s):
                """qproj + q8 for one slice. q8 slice s is first read when
                block 2s hoists, so slices 1-3 run in blocks 1/3/5 where the
                pipeline has slack (instead of crowding block 0)."""
                lo = s * SLICE
                with nc.allow_low_precision(reason="fp8 attention, tol 2e-2"):
                    ps = ps_sc.tile([C, 2, 512], f32, tag="sc", name=f"qps{s}")
                    for h in range(2):
                        hsl = slice(lo + h * 512, lo + (h + 1) * 512)
                        nc.tensor.matmul(ps[:, h, :], wqt16[:], xf16[:, hsl],
                                         start=True, stop=True)
                    nc.vector.tensor_scalar(q8z[:, 1, lo:lo + SLICE],
                                            ps[:, :, :], bq, None, op0=ADD)

            def resid(s):
                """xfb += bv for one slice (after the slice's cast read xfb)."""
                sl = slice(s * SLICE, (s + 1) * SLICE)
                nc.vector.tensor_scalar(xfb[:, sl], xfb[:, sl], bv, None,
                                        op0=ADD)

            def setup_v(s):
                """v8 chunks for one 1024-col slice (8 chunks, one psum tile)."""
                psv = ps_sc.tile([C, 2, 512], f32, tag="sc", name=f"vps{s}")
                for t in range(8):
                    j = s * 8 + t
                    nc.tensor.matmul(psv[:, t // 4, (t % 4) * C:(t % 4 + 1) * C],
                                     xf16[:, j * C:(j + 1) * C], wvt16[:],
                                     start=True, stop=True)
                with nc.allow_low_precision(reason="fp8 attention, tol 2e-2"):
                    nc.vector.tensor_copy(v8[:, s * 8:(s + 1) * 8, :], psv[:, :, :])

            def emit_qk_exp(nb, jp, on_dve):
                """QK DoubleRow matmuls + exp for one chunk-pair."""
                n0 = nb * NB
                sc = ps_sc.tile([C, 2, 512], f32, tag="sc", name=f"sc{nb}_{jp}")
                for u in range(2):
                    # moving planes: u=0 -> [q8, 0], u=1 -> [0, q8]
                    nc.tensor.matmul(sc[:, u, :], k8[:, 2 * jp:2 * jp + 2, :],
                                     q8z[:, 1 - u:3 - u, n0:n0 + NB],
                                     start=True, stop=True, perf_mode=DR)
                es = es_pool.tile([C, 2, 512], f8, tag="es", name=f"es{nb}_{jp}")
                with nc.allow_low_precision(reason="fp8 attention, tol 2e-2"):
                    if on_dve:
                        nc.vector.tensor_scalar(es[:, :, :].bitcast(u8),
                                                sc[:, :, :], float(BH_A),
                                                float(BH_B), op0=MUL, op1=ADD)
                    else:
                        nc.scalar.activation(es[:, :, :], sc[:, :, :], EXP,
                                             bias=ebias[:], scale=float(SCALE))
                return es

            def emit_pv_dn(nb, jp, pv, dn, es):
                nc.tensor.matmul(pv[:], v8[:, 2 * jp:2 * jp + 2, :], es[:, :, :],
                                 start=(jp == 0), stop=(jp == NPAIR - 1),
                                 perf_mode=DR)
                nc.tensor.matmul(dn[:], ones8[:, :, :], es[:, :, :],
                                 start=(jp == 0), stop=(jp == NPAIR - 1),
                                 perf_mode=DR)

            def tail(nb, pv, dn):
                osl = slice(nb * NB, (nb + 1) * NB)
                rb = work.tile([C, NB], f32, tag="rb", name=f"rb{nb}")
                ep1 = work.tile([C, NB], f32, tag="ep1", name=f"ep{nb}")
                ost = work.tile([C, NB], f32, tag="ost", name=f"ost{nb}")
                if nb == NBLK - 1:
                    # final tail: no successor needs the psum banks, so skip
                    # the pv drain and pipeline the whole normalize + DMA in
                    # 256-col halves (first half's DMA overlaps second half)
                    for h in (slice(0, 256), slice(256, NB)):
                        o2 = slice(nb * NB + h.start, nb * NB + h.stop)
                        nc.vector.reciprocal_approx_fast(rb[:, h], dn[:, h])
                        nc.vector.tensor_tensor(ep1[:, h], pv[:, h], rb[:, h],
                                                op=MUL)
                        nc.vector.tensor_tensor(ost[:, h], ep1[:, h],
                                                xfb[:, o2], op=ADD)
                        nc.sync.dma_start(out_d[:, o2], ost[:, h])
                    return
                pvs = work.tile([C, NB], f32, tag="pvs", name=f"pvs{nb}")
                # drain pv to SBUF FIRST: unlike the recip->mul chain it has
                # no dependencies, so the next block's first PV (WAR on the
                # single pv psum bank) unblocks ~1us sooner
                nc.vector.tensor_copy(pvs[:], pv[:])
                nc.vector.reciprocal_approx_fast(rb[:], dn[:])
                nc.vector.tensor_tensor(ep1[:], pvs[:], rb[:], op=MUL)
                nc.gpsimd.tensor_tensor(ost[:], ep1[:], xfb[:, osl], op=ADD)
                nc.sync.dma_start(out_d[:, osl], ost[:])

            # quarter-granular first setup: k8 chunks 0-3 land pair-0/1
            # stationaries the moment each 256-col x quarter arrives
            with nc.allow_low_precision(reason="fp8 attention, tol 2e-2"):
                nc.vector.tensor_copy(xf16[:, 0:256], xfb[:, 0:256])
                nc.vector.tensor_scalar(k8[:, 0:2, :], xf16[:, 0:256],
                                        wks, bk, op0=MUL, op1=ADD)
                nc.vector.tensor_copy(xf16[:, 256:512], xfb[:, 256:512])
                nc.vector.tensor_scalar(k8[:, 2:4, :], xf16[:, 256:512],
                                        wks, bk, op0=MUL, op1=ADD)
                ps0 = ps_sc.tile([C, 2, 512], f32, tag="sc", name="qps0_0")
                nc.tensor.matmul(ps0[:, 0, :], wqt16[:], xf16[:, 0:512],
                                 start=True, stop=True)
                nc.vector.tensor_scalar(q8z[:, 1, 0:512], ps0[:, 0:1, :],
                                        bq, None, op0=ADD)
            with nc.allow_low_precision(reason="fp8 attention, tol 2e-2"):
                nc.vector.tensor_copy(xf16[:, 512:SLICE], xfb[:, 512:SLICE])
                nc.vector.tensor_scalar(k8[:, 4:8, :], xf16[:, 512:SLICE],
                                        wks, bk, op0=MUL, op1=ADD)
                ps1 = ps_sc.tile([C, 2, 512], f32, tag="sc", name="qps0_1")
                nc.tensor.matmul(ps1[:, 0, :], wqt16[:], xf16[:, 512:SLICE],
                                 start=True, stop=True)
                nc.vector.tensor_scalar(q8z[:, 1, 512:SLICE], ps1[:, 0:1, :],
                                        bq, None, op0=ADD)

            hoisted = {}
            for nb in range(NBLK):
                dv = DV0 if nb == 0 else (DV7 if nb == NBLK - 1 else DV)
                pv = ps_pv.tile([C, NB], f32, tag="pv", name=f"pv{nb}")
                dn = ps_dn.tile([C, NB], f32, tag="dn", name=f"dn{nb}")
                pending = []

                for jp in range(NPAIR):
                    # spread the projection setups so no single burst starves
                    # the QK->exp stream (qk at 2/6/10, v at 4/8/12)
                    if nb == 0 and jp in (2, 6, 10):
                        setup_k((jp + 2) // 4)  # slices 1, 2, 3
                        if jp == 2:  # zero planes for blocks 4-7
                            nc.gpsimd.memset(q8z[:, 2, 2048:N], 0.0)
                            nc.gpsimd.memset(q8z[:, 0, 2048:N], 0.0)
                    if nb == 0 and jp in (4, 8, 12):
                        setup_v(jp // 4)
                        resid(jp // 4)
                    if jp == 2 and nb in (1, 3, 5):
                        setup_q((nb + 1) // 2)  # slices 1, 2, 3
                    es = hoisted.pop((nb, jp), None)
                    if es is None:
                        es = emit_qk_exp(nb, jp, jp in dv)
                    if nb == 0 and jp == 0:
                        setup_v(0)
                        resid(0)
                    pending.append((jp, es))
                    if jp == NPAIR - 1:
                        # boundary: flush all but the final pair (their exps
                        # are long done), hoist 2 QK/exps, flush the final
                        # pair (its exp has now had time to finish), then 2
                        # more hoists -- the PE never waits on a fresh exp
                        # and ACT stays fed through the tail
                        while len(pending) > 1:
                            pjp, pes = pending.pop(0)
                            emit_pv_dn(nb, pjp, pv, dn, pes)
                        if nb < NBLK - 1:
                            for hj in range(2):
                                hoisted[(nb + 1, hj)] = emit_qk_exp(nb + 1, hj, False)
                        pjp, pes = pending.pop(0)
                        emit_pv_dn(nb, pjp, pv, dn, pes)
                        if nb < NBLK - 1:
                            for hj in range(2, NHOIST):
                                hoisted[(nb + 1, hj)] = emit_qk_exp(nb + 1, hj, False)
                    elif len(pending) > LAG:
                        pjp, pes = pending.pop(0)
                        emit_pv_dn(nb, pjp, pv, dn, pes)

                tail(nb, pv, dn)

    nc.finalize()
    return nc


def _get_nc():
    if "nc" not in _cache:
        _cache["nc"] = _build_nc()
    return _cache["nc"]


def make_in_maps(x, Wq, bq, Wk, bk, Wv, bv):
    x = np.asarray(x, dtype=np.float32)
    B = x.shape[0]
    wqt16 = np.ascontiguousarray(np.asarray(Wq, np.float32).T).astype(np.float16)
    wvt16 = np.ascontiguousarray(np.asarray(Wv, np.float32).T).astype(np.float16)
    w4 = np.ascontiguousarray(np.stack([
        np.asarray(Wk, np.float32).sum(axis=0),
        np.asarray(bk, np.float32),
        np.asarray(bq, np.float32),
        np.asarray(bv, np.float32),
    ], axis=1))
    in_maps = []
    for i in range(B):
        in_maps.append({
            "x": np.ascontiguousarray(x[i].reshape(C, N)),
            "wqt16": wqt16, "wvt16": wvt16, "w4": w4,
        })
    return in_maps


def kernel(x, Wq, bq, Wk, bk, Wv, bv, _trace=False, _tmpdir=None):
    from concourse.bass_utils import run_bass_kernel_spmd

    x = np.asarray(x, dtype=np.float32)
    B, c, H, W = x.shape
    assert (c, H * W) == (C, N), (c, H, W)
    in_maps = make_in_maps(x, Wq, bq, Wk, bk, Wv, bv)
    nc = _get_nc()
    res = run_bass_kernel_spmd(nc, in_maps, core_ids=list(range(B)),
                               trace=_trace, tmpdir=_tmpdir)
    out = np.stack([res.results[i]["out"].reshape(C, H, W) for i in range(B)])
    if _trace:
        _cache["last_result"] = res
    return out.astype(np.float32)
